# revision 19
# baseline (speedup 1.0000x reference)
"""AGCN Trainium2 kernel — 8-core data-parallel over batch.

Math (per batch b, N=1000 nodes, Din=32, Dout=64, D=16, K=2):
  AS  = relu(tanh(3 E E^T))                       [N,N] static, symmetric
  d   = rowsum(AS)^-1/2 ; AG = d AS d
  ho  = AS @ x[b]; DF = MLP(ho); Et = E*TD*TW; DE = tanh(3 Et DF)
  DA  = relu(tanh(3 DE DE^T))                     [N,N] per batch
  out = a*(einsum over per-node weights E@Wpool applied to [x, AG@x]) + a*E@bias_pool
      + b*(DA @ x) @ Wdg

Each core handles 4 batches; AS/weights replicated per core.
v2 notes:
  - inputs packed into 2 DRAM tensors (3+1 descriptors) to kill
    DIRECT2D descriptor-issue serialization at startup.
  - E64 (E rows broadcast 64-wide for the agconv yt products) is built
    with ONE broadcast DMA from the packed DRAM input at kernel start;
    the d factor is no longer folded into E but applied on the xg
    drain via a d128 broadcast tile (tiny DRAM round-trip of d only).
  - AS / bias / S matmuls sliced to their true contraction depth K=16
    (no zero-row streaming); everything bf16 on the PE.
  - MLP sigmoids replaced by tanh identities (sigmoid(z) =
    0.5*tanh(z/2)+0.5) with affine corrections folded into the next
    layer's weights host-side -> single ACT table, no table loads.
  - output DMA: 2 descriptors per batch instead of 8, spread across
    engines.
"""

import os
import sys

for _p in ("/opt/trn_rl_repo", "/root/.axon_site/_ro/trn_rl_repo"):
    if os.path.isdir(_p) and _p not in sys.path:
        sys.path.append(_p)

from contextlib import ExitStack

import ml_dtypes
import numpy as np

import concourse.bass as bass
import concourse.tile as tile
from concourse import bacc, mybir
from concourse.masks import make_identity

BF16 = mybir.dt.bfloat16
F32 = mybir.dt.float32
I32 = mybir.dt.int32
AF = mybir.ActivationFunctionType
OP = mybir.AluOpType
bfloat16 = ml_dtypes.bfloat16

NCORES = 8
NB = 4  # batches per core
N = 1000
NP = 1024
NT = 8  # node tiles of 128
ALPHA = 3.0
RSQRT_MAGIC = 0x5F3759DF

LAST_EXEC_NS = None
_NC_CACHE = {}

# packed bf16 input layout (columns)
C_XQ = 0          # [128, 8, 128]
C_XT2 = 1024      # [128, 4, 1024]
C_ETB = 5120      # [128, 1024]
C_TDT = 6144
C_TWT = 7168
C_WPT = 8192      # [128, 8, 64]
C_FC1 = 8704      # [128, 32]
C_FC2 = 8736      # [128, 16]
C_FC3 = 8752      # [128, 16]
C_BWDG = 8768     # [128, 64]
C_AB = 8832       # [16, 64] (rows 0-15)
C16 = 8896
# packed f32 input layout
F_B1 = 0
F_B2 = 1
F_B3 = 2
F_DM = 3          # [128, 8]
C32 = 12

if os.environ.get("KERNEL_LDW_OPT", "0") == "1":
    from concourse import bass_utils as _bu

    _orig_run_command = _bu.run_command

    def _run_command_ldw(argv, **kw):
        argv = [a.replace("--enable-ldw-opt=false", "--enable-ldw-opt=true")
                if isinstance(a, str) else a for a in argv]
        return _orig_run_command(argv, **kw)

    _bu.run_command = _run_command_ldw


def _build_body(nc, tc, ctx, t):
    """t: dict of dram tensor handles."""
    P = 128
    PHASES = int(os.environ.get("KERNEL_PHASES", "6"))

    pers = ctx.enter_context(tc.tile_pool(name="pers", bufs=1))
    work = ctx.enter_context(tc.tile_pool(name="work", bufs=3))
    da_p = ctx.enter_context(tc.tile_pool(name="da", bufs=6))
    yt_p = ctx.enter_context(tc.tile_pool(name="yt", bufs=6))
    # PSUM static budget: 8 banks = 16KB/partition.
    # ps_b tag "big" bufs=2 -> 4 banks: AS, MLP, S tiles (sequential uses)
    # ps_a bufs=1, tag acc1 (ho -> dg -> tr) 2 banks, tag acc2 (xg -> ag) 2 banks
    ps_b = ctx.enter_context(tc.tile_pool(name="ps_b", bufs=2, space="PSUM"))
    ps_a = ctx.enter_context(tc.tile_pool(name="ps_a", bufs=1, space="PSUM"))
    dram = ctx.enter_context(tc.tile_pool(name="dram", bufs=1, space="DRAM"))

    # ---- persistent SBUF tiles ----
    pkb = pers.tile([P, C16], BF16)
    pkf = pers.tile([P, C32], F32)
    xq = pkb[:, C_XQ:C_XQ + 1024].rearrange("p (t c) -> p t c", t=NT)
    xT2 = pkb[:, C_XT2:C_XT2 + 4096].rearrange("p (b c) -> p b c", b=NB)
    ETb = pkb[:, C_ETB:C_ETB + 1024]
    TDT = pkb[:, C_TDT:C_TDT + 1024]
    TWT = pkb[:, C_TWT:C_TWT + 1024]
    WpT = pkb[:, C_WPT:C_WPT + 512].rearrange("p (t c) -> p t c", t=NT)
    fc1T = pkb[:, C_FC1:C_FC1 + 32]
    fc2T = pkb[:, C_FC2:C_FC2 + 16]
    fc3T = pkb[:, C_FC3:C_FC3 + 16]
    bWdg4 = pkb[:, C_BWDG:C_BWDG + 64]
    ab16 = pkb[0:16, C_AB:C_AB + 64]
    b1p = pkf[:, F_B1:F_B1 + 1]
    b2p = pkf[:, F_B2:F_B2 + 1]
    b3p = pkf[:, F_B3:F_B3 + 1]
    dmask = pkf[:, F_DM:F_DM + 8]

    E64 = pers.tile([P, NT, NP], BF16)        # chunk c: rows 0-63 E[2c], 64-127 E[2c+1]
    dxq = pers.tile([P, NT, P], BF16)
    AS = pers.tile([P, NT, NP], BF16)         # AS row-tiles
    hoT = pers.tile([P, NP], BF16)
    h1 = pers.tile([P, NP], BF16)
    h2 = pers.tile([P, NP], BF16)
    Et = pers.tile([P, NP], BF16)
    EtDF = pers.tile([P, NP], BF16)
    DET = pers.tile([P, NP], BF16)
    dgT = pers.tile([P, NP], BF16)            # x_dg2^T packed (4b*32c)
    xg_sb = pers.tile([P, NP], BF16)          # staging for d*(AS@(d*x)) drains
    d128 = pers.tile([P, NP], BF16)           # d broadcast along free dim
    ones1 = pers.tile([1, P], BF16)
    agT = pers.tile([P, 2, NP], BF16)         # final out^T per batch-pair
    I128f = pers.tile([P, P], F32)
    I128b = pers.tile([P, P], BF16)
    rs_col = pers.tile([P, NT], F32)          # rowsum(AS) per node tile
    biasT2 = pers.tile([P, NP], F32)
    d_col = pers.tile([P, NT], F32)
    magic = pers.tile([P, NT], I32)

    # ---- input DMAs: ETb gates AS, xq gates ho -- both early, balanced
    # across the two hwdge rings; xT2/weights land later.
    nc.sync.dma_start(out=pkb[:, C_ETB:C_ETB + 1024],
                      in_=t["pkb"].ap()[:, C_ETB:C_ETB + 1024])
    nc.sync.dma_start(out=pkb[:, 0:1024], in_=t["pkb"].ap()[:, 0:1024])
    nc.sync.dma_start(out=pkb[:, 1024:3072], in_=t["pkb"].ap()[:, 1024:3072])
    nc.scalar.dma_start(out=pkb[:, 3072:5120], in_=t["pkb"].ap()[:, 3072:5120])
    nc.scalar.dma_start(out=pkb[:, C_TDT:C16], in_=t["pkb"].ap()[:, C_TDT:C16])
    nc.gpsimd.dma_start(out=pkf, in_=t["pkf"].ap())

    make_identity(nc, I128f)
    make_identity(nc, I128b)
    nc.vector.memset(magic, RSQRT_MAGIC)
    nc.vector.memset(ones1, 1.0)

    out_d = t["out"]
    oa = out_d.ap()
    dma_engs = [nc.sync, nc.scalar, nc.gpsimd]

    # ================= Phase 1: AS = relu(tanh(3 E E^T)) =================
    # K=16 contraction (single E^T replica rows 0-15); rowsums fused into
    # the per-tile loop so the d-chain starts as early as possible.
    for mt in range(NT):
        ps = ps_b.tile([P, NP], F32, tag="big")
        for r in range(2):
            nc.tensor.matmul(
                ps[:, r * 512:(r + 1) * 512],
                lhsT=ETb[0:16, mt * P:(mt + 1) * P],
                rhs=ETb[0:16, r * 512:(r + 1) * 512],
                start=True, stop=True,
            )
        nc.scalar.activation(AS[:, mt, :], ps, AF.Tanh, scale=ALPHA)
        nc.vector.tensor_scalar_max(AS[:, mt, :], AS[:, mt, :], 0.0)
        nc.vector.tensor_reduce(rs_col[:, mt:mt + 1], AS[:, mt, :],
                                mybir.AxisListType.X, OP.add)

    # ho matmuls depend only on AS -- keep PE busy during the d-chain
    ho_ps = ps_a.tile([P, NP], F32, tag="acc1")
    xg_ps = ps_a.tile([P, NP], F32, tag="acc2")
    for vt in range(NT):
        for r in range(2):
            nc.tensor.matmul(
                ho_ps[:, r * 512:(r + 1) * 512],
                lhsT=xq[:, vt, :],
                rhs=AS[:, vt, r * 512:(r + 1) * 512],
                start=(vt == 0), stop=(vt == NT - 1),
            )
    # hoT drain early so m1 can run right after ho
    nc.vector.tensor_copy(hoT, ho_ps)

    # E64 broadcast: gated on an AS corner via a WAW dep so its 2MB
    # transfer does not compete with the input loads.
    nc.vector.tensor_copy(E64[0:1, 0, 0:8], AS[0:1, 0, 0:8])
    pk = t["pkb"].ap()
    for hf in range(2):
        e_src = bass.AP(
            tensor=pk.tensor, offset=pk.offset + C_ETB + hf * C16,
            ap=[[0, 64], [2 * C16, NT], [1, NP]])
        nc.gpsimd.dma_start(out=E64[64 * hf:64 * hf + 64, :, :], in_=e_src)

    # rsqrt via magic-number + 2 Newton iterations (avoids ACT table switch)
    s_sb = work.tile([P, NT], F32, tag="dtmp")
    nc.vector.tensor_scalar_max(s_sb, rs_col, 1e-6)
    sh = work.tile([P, NT], I32, tag="dtmp_i")
    nc.vector.tensor_scalar(sh, s_sb.bitcast(I32), 1, 0, OP.logical_shift_right, OP.bypass)
    y0 = work.tile([P, NT], I32, tag="dtmp_y")
    nc.vector.tensor_tensor(y0, magic, sh, OP.subtract)
    yf = y0.bitcast(F32)
    cur = yf
    for it in range(2):
        t1 = work.tile([P, NT], F32, tag=f"nt1_{it}")
        nc.vector.tensor_tensor(t1, cur, cur, OP.mult)
        nc.vector.tensor_tensor(t1, t1, s_sb, OP.mult)
        nc.vector.tensor_scalar(t1, t1, -0.5, 1.5, OP.mult, OP.add)
        t2 = work.tile([P, NT], F32, tag=f"nt2_{it}")
        nc.vector.tensor_tensor(t2, cur, t1, OP.mult)
        cur = t2
    nc.vector.tensor_tensor(d_col, cur, dmask, OP.mult)  # mask kills padded nodes

    # dxq = d * x (token-major, per-partition scalar)
    for mt in range(NT):
        nc.vector.tensor_scalar_mul(dxq[:, mt, :], xq[:, mt, :], d_col[:, mt:mt + 1])

    # ===================== MLP m1 (tanh-folded) =====================
    # sigmoid(z) = 0.5*tanh(z/2)+0.5; affine parts folded into fc2/fc3
    # host-side, so only Tanh is ever used (no ACT table switches).
    m1_ps = ps_b.tile([P, NP], F32, tag="big")
    for j in range(NB):
        for r in range(2):
            nc.tensor.matmul(
                m1_ps[32 * j:32 * j + 32, r * 512:(r + 1) * 512],
                lhsT=fc1T[32 * j:32 * j + 32, :],
                rhs=hoT[32 * j:32 * j + 32, r * 512:(r + 1) * 512],
                start=True, stop=True, tile_position=(32 * j, 32 * j),
            )
    nc.scalar.activation(h1, m1_ps, AF.Tanh, bias=b1p[:, 0:1], scale=0.5)

    # biasT2[0:64]=a*(E@bias_pool)^T, [64:128]=same (both batches of a pair)
    bias_ps = ps_b.tile([P, NP], F32, tag="big")
    for half in range(2):
        for tch in range(2):
            nc.tensor.matmul(
                bias_ps[64 * half:64 * half + 64, tch * 512:(tch + 1) * 512],
                lhsT=ab16, rhs=ETb[0:16, tch * 512:(tch + 1) * 512],
                start=True, stop=True, tile_position=(0, 64 * half),
            )

    # d128: d as a free-dim row broadcast to all 128 partitions. A tiny
    # DRAM round-trip (2KB) turns the per-partition d_col into a [1,1024]
    # row: write [128,8] bf16 row-major, read back with a gather AP
    # (n = mt*128+p -> offset p*8+mt); then a ones[1,128] x row matmul
    # broadcasts it to every partition.
    d_colb = work.tile([P, NT], BF16, tag="d8")
    nc.vector.tensor_copy(d_colb, d_col)
    dcol_d = dram.tile([P, NT], BF16)
    nc.gpsimd.dma_start(out=dcol_d, in_=d_colb)
    d8r = work.tile([1, NP], BF16, tag="d8r")
    dsrc = bass.AP(tensor=dcol_d.tensor, offset=dcol_d.offset,
                   ap=[[1, NT], [NT, P]])
    nc.gpsimd.dma_start(out=d8r.rearrange("o (t p) -> o t p", t=NT), in_=dsrc)
    d_bc = ps_b.tile([P, NP], F32, tag="big")
    for r in range(2):
        nc.tensor.matmul(d_bc[:, r * 512:(r + 1) * 512], lhsT=ones1,
                         rhs=d8r[0:1, r * 512:(r + 1) * 512],
                         start=True, stop=True)
    nc.vector.tensor_copy(d128, d_bc)

    if PHASES < 2:
        probe = work.tile([P, NT, 64], F32, tag="probe")
        for mt in range(NT):
            nc.vector.tensor_copy(probe[:, mt, :], AS[:, mt, 0:64])
        for nt_i in range(NT):
            nc.sync.dma_start(out=out_d.ap()[0, nt_i * 125:nt_i * 125 + 125, :],
                              in_=probe[0:125, nt_i, :])
        nc.sync.dma_start(out=out_d.ap()[1, 0:128, 0:8],
                          in_=d_col)
        p2 = work.tile([P, 64], F32, tag="probe2")
        nc.vector.tensor_copy(p2, E64[:, 0, 0:64])
        nc.sync.dma_start(out=out_d.ap()[1, 128:256, 0:64], in_=p2)
        p3 = work.tile([P, 64], F32, tag="probe3")
        nc.vector.tensor_copy(p3, d128[:, 0:64])
        nc.sync.dma_start(out=out_d.ap()[1, 256:384, 0:64], in_=p3)
        return

    # ============ Phase 2: xg = d * (AS @ (d*x))^T ============
    for vt in range(NT):
        for r in range(2):
            nc.tensor.matmul(
                xg_ps[:, r * 512:(r + 1) * 512],
                lhsT=dxq[:, vt, :],
                rhs=AS[:, vt, r * 512:(r + 1) * 512],
                start=(vt == 0), stop=(vt == NT - 1),
            )

    # ===================== MLP m2/m3 =====================
    m2_ps = ps_b.tile([P, NP], F32, tag="big")
    for j in range(NB):
        for r in range(2):
            nc.tensor.matmul(
                m2_ps[32 * j:32 * j + 16, r * 512:(r + 1) * 512],
                lhsT=fc2T[32 * j:32 * j + 32, :],
                rhs=h1[32 * j:32 * j + 32, r * 512:(r + 1) * 512],
                start=True, stop=True, tile_position=(32 * j, 32 * j),
            )
    nc.scalar.activation(h2, m2_ps, AF.Tanh, bias=b2p[:, 0:1], scale=0.5)
    m3_ps = ps_b.tile([P, NP], F32, tag="big")
    for j in range(NB):
        for r in range(2):
            nc.tensor.matmul(
                m3_ps[32 * j:32 * j + 16, r * 512:(r + 1) * 512],
                lhsT=fc3T[32 * j:32 * j + 16, :],
                rhs=h2[32 * j:32 * j + 16, r * 512:(r + 1) * 512],
                start=True, stop=True, tile_position=(32 * j, 32 * j),
            )

    # drains not on the DET critical path
    nc.vector.tensor_tensor(xg_sb, xg_ps, d128, OP.mult)
    for j in range(NB):
        dma_engs[j % 3].dma_start(out=xT2[32:64, j, :], in_=xg_sb[32 * j:32 * j + 32, :])
        dma_engs[(j + 1) % 3].dma_start(out=xT2[96:128, j, :], in_=xg_sb[32 * j:32 * j + 32, :])
    nc.vector.tensor_copy(biasT2, bias_ps)
    nc.vector.tensor_tensor(Et, TDT, TWT, OP.mult)
    nc.vector.tensor_tensor(Et, Et, ETb, OP.mult)

    if PHASES < 4:
        nc.vector.scalar_tensor_tensor(EtDF, m3_ps, b3p[:, 0:1], Et, OP.add, OP.mult)
        nc.scalar.activation(DET, EtDF, AF.Tanh, scale=ALPHA)
        p2 = work.tile([P, 64], F32, tag="probe2")
        nc.vector.tensor_copy(p2, DET[:, 0:64])
        nc.sync.dma_start(out=out_d.ap()[0, 0:128, :], in_=p2)
        p3 = work.tile([P, 64], F32, tag="probe3")
        nc.vector.tensor_copy(p3, xT2[:, 0, 0:64])
        nc.sync.dma_start(out=out_d.ap()[1, 0:128, :], in_=p3)
        return

    # ===== Phase 4a: agconv z chunks. For pair 0 (inline, pre-S) the
    # EtDF/DET ops are slotted after chunk 1's yt builds so DET is ready
    # the moment the PE finishes the z0 block.
    def z_chunks(zp, ag_ps, cs, hook=None):
        for c in cs:
            for bb in range(2):
                j = 2 * zp + bb
                yt = yt_p.tile([P, NP], BF16, tag="yt")
                nc.vector.tensor_tensor(yt, xT2[:, j, :], E64[:, c, :], OP.mult)
                for tch in range(2):
                    nc.tensor.matmul(
                        ag_ps[64 * bb:64 * bb + 64, tch, :],
                        lhsT=WpT[:, c, :],
                        rhs=yt[:, tch * 512:(tch + 1) * 512],
                        start=(c == 0), stop=(c == NT - 1),
                        tile_position=(0, 64 * bb),
                        skip_group_check=True,
                    )
            if hook is not None and c == 1:
                hook()

    def ag_drain(zp, ag_ps):
        nc.vector.tensor_tensor(
            agT[:, zp, :].rearrange("p (a b) -> p a b", a=2),
            ag_ps, biasT2.rearrange("p (a b) -> p a b", a=2), OP.add)

    def det_hook():
        # EtDF = (DF + b3) * Et ; DE^T = tanh(3 EtDF)
        nc.vector.scalar_tensor_tensor(EtDF, m3_ps, b3p[:, 0:1], Et, OP.add, OP.mult)
        nc.scalar.activation(DET, EtDF, AF.Tanh, scale=ALPHA)

    ag0 = ps_a.tile([P, 2, 512], F32, tag="acc2")
    z_chunks(0, ag0, range(NT), hook=det_hook)
    ag_drain(0, ag0)

    # ===== Phase 4b: S / DA / x_dg2 pipeline, z1 interleaved on mt 4-7 =====
    dg_ps = ps_a.tile([P, NP], F32, tag="acc1")
    ag1 = None
    for mt in range(NT):
        s_tiles = []
        da_tiles = []
        for half in range(2):
            # two S tiles in flight (psum big pool bufs=2); K=16 slices of DET
            for bb in range(2):
                j = 2 * half + bb
                s_ps = ps_b.tile([P, NP], F32, tag="big")
                s_tiles.append(s_ps)
                for r in range(2):
                    nc.tensor.matmul(
                        s_ps[:, r * 512:(r + 1) * 512],
                        lhsT=DET[32 * j:32 * j + 16, mt * P:(mt + 1) * P],
                        rhs=DET[32 * j:32 * j + 16, r * 512:(r + 1) * 512],
                        start=True, stop=True, tile_position=(32 * j, 0),
                    )
            for bb in range(2):
                j = 2 * half + bb
                da_t = da_p.tile([P, NP], BF16, tag="da")
                da_tiles.append(da_t)
                nc.scalar.activation(da_t, s_tiles[j], AF.Tanh, scale=ALPHA)
                nc.vector.tensor_scalar_max(da_t, da_t, 0.0)
            for bb in range(2):
                j = 2 * half + bb
                for r in range(2):
                    nc.tensor.matmul(
                        dg_ps[32 * j:32 * j + 32, r * 512:(r + 1) * 512],
                        lhsT=xq[:, mt, 32 * j:32 * j + 32],
                        rhs=da_tiles[j][:, r * 512:(r + 1) * 512],
                        start=(mt == 0), stop=(mt == NT - 1),
                        tile_position=(0, 32 * j),
                    )
        if mt >= 4:
            if ag1 is None:
                ag1 = ps_a.tile([P, 2, 512], F32, tag="acc2")
            z_chunks(1, ag1, (2 * (mt - 4), 2 * (mt - 4) + 1))
    ag_drain(1, ag1)
    nc.vector.tensor_copy(dgT, dg_ps)

    # ====== Phase 6: transpose to token-major + dgconv fold + DMA out ======
    # bf16 transposes (1 cycle/row) into a bf16 psum; dg fold into a f32
    # psum; fold the two on the DVE during the drain.
    for b in range(NB):
        pair, bb = b // 2, b % 2
        trt = ps_a.tile([P, NT, 64], BF16, tag="acc1" if b % 2 == 0 else "acc2")
        trd = ps_b.tile([P, NT, 64], F32, tag="big")
        for nt_i in range(NT):
            nc.tensor.matmul(
                trt[:, nt_i, :],
                lhsT=agT[64 * bb:64 * bb + 64, pair, nt_i * P:(nt_i + 1) * P],
                rhs=I128b[64 * bb:64 * bb + 64, 64 * bb:64 * bb + 64],
                is_transpose=True, start=True, stop=True,
            )
            nc.tensor.matmul(
                trd[:, nt_i, :],
                lhsT=dgT[32 * b:32 * b + 32, nt_i * P:(nt_i + 1) * P],
                rhs=bWdg4[32 * b:32 * b + 32, :],
                start=True, stop=True,
                tile_position=(32 * b, 0),
                skip_group_check=True,
            )
        trt_sb = work.tile([P, NT, 64], BF16, tag=f"trt_sb{b % 2}")
        nc.scalar.copy(trt_sb, trt)
        tr_sb = work.tile([P, NT, 64], F32, tag=f"tr_sb{b % 2}")
        nc.vector.tensor_tensor(tr_sb, trd, trt_sb, OP.add)
        # 2 output descriptors per batch: tiles 0-6 in one 3D AP, tile 7 alone
        o_ap = bass.AP(tensor=oa.tensor, offset=oa.offset + b * N * 64,
                       ap=[[64, 128], [128 * 64, 7], [1, 64]])
        dma_engs[b % 2].dma_start(out=o_ap, in_=tr_sb[:, 0:7, :])
        o_ap7 = bass.AP(tensor=oa.tensor,
                        offset=oa.offset + b * N * 64 + 896 * 64,
                        ap=[[64, 104], [1, 64]])
        dma_engs[(b + 1) % 2].dma_start(out=o_ap7, in_=tr_sb[0:104, 7, :])


def _build_nc():
    nc = bacc.Bacc("TRN2", target_bir_lowering=False, debug=False,
                   num_devices=NCORES)
    P = 128
    t = {}
    t["pkb"] = nc.dram_tensor("pkb", [P, C16], BF16, kind="ExternalInput")
    t["pkf"] = nc.dram_tensor("pkf", [P, C32], F32, kind="ExternalInput")
    t["out"] = nc.dram_tensor("out", [NB, N, 64], F32, kind="ExternalOutput")

    with tile.TileContext(nc) as tc:
        with ExitStack() as ctx:
            _build_body(nc, tc, ctx, t)
    nc.finalize()
    return nc


def _prep_core_inputs(core, x, E, TD, TW, Wp, bp, Wdg, a, b,
                      fc1_w, fc1_b, fc2_w, fc2_b, fc3_w, fc3_b):
    P = 128
    bs = slice(NB * core, NB * (core + 1))
    xp = np.zeros((NB, NP, 32), np.float32)
    xp[:, :N] = x[bs]
    Ep = np.zeros((NP, 16), np.float32)
    Ep[:N] = E

    pkb = np.zeros((P, C16), np.float32)
    pkf = np.zeros((P, C32), np.float32)

    xq = pkb[:, C_XQ:C_XQ + 1024].reshape(P, NT, P)
    for ti in range(NT):
        blk = xp[:, ti * P:(ti + 1) * P, :]          # [4,128,32]
        xq[:, ti, :] = blk.transpose(1, 0, 2).reshape(P, P)
    xT2 = pkb[:, C_XT2:C_XT2 + 4096].reshape(P, NB, NP)
    xT = xp.transpose(2, 0, 1)                        # [32, 4, 1024]
    xT2[0:32] = xT
    xT2[64:96] = xT

    # MLP folding: sigmoid(z) = 0.5*tanh(z/2)+0.5
    #  h1 = 0.5*t1+0.5, t1 = tanh(0.5*(fc1 ho + b1))
    #  z2 = fc2 h1 + b2 = (0.5 fc2) t1 + (b2 + 0.5 rowsum(fc2))
    #  h2 = 0.5*t2+0.5, t2 = tanh(0.5*z2)
    #  DF = fc3 h2 + b3 = (0.5 fc3) t2 + (b3 + 0.5 rowsum(fc3))
    fc2h = 0.5 * fc2_w
    b2f = fc2_b + 0.5 * fc2_w.sum(axis=1)
    fc3h = 0.5 * fc3_w
    b3f = fc3_b + 0.5 * fc3_w.sum(axis=1)

    for j in range(NB):
        r0 = 32 * j
        pkb[r0:r0 + 16, C_ETB:C_ETB + 1024] = Ep.T
        pkb[r0:r0 + 16, C_TDT:C_TDT + 1024][:, :N] = TD[NB * core + j].T
        pkb[r0:r0 + 16, C_TWT:C_TWT + 1024][:, :N] = TW[NB * core + j].T
        pkb[r0:r0 + 32, C_FC1:C_FC1 + 32] = fc1_w.T
        pkb[r0:r0 + 32, C_FC2:C_FC2 + 16] = fc2h.T
        pkb[r0:r0 + 16, C_FC3:C_FC3 + 16] = fc3h.T
        pkb[r0:r0 + 32, C_BWDG:C_BWDG + 64] = b * Wdg
        pkf[r0:r0 + 32, F_B1] = 0.5 * fc1_b
        pkf[r0:r0 + 16, F_B2] = 0.5 * b2f
        pkf[r0:r0 + 16, F_B3] = b3f

    for mt in range(NT):
        for p in range(P):
            pkf[p, F_DM + mt] = 1.0 if mt * P + p < N else 0.0

    WpT = pkb[:, C_WPT:C_WPT + 512].reshape(P, NT, 64)
    for c in range(NT):
        for h in range(2):
            d = 2 * c + h
            WpT[64 * h:64 * h + 32, c, :] = a * Wp[d, 0]
            WpT[64 * h + 32:64 * h + 64, c, :] = a * Wp[d, 1]

    pkb[0:16, C_AB:C_AB + 64] = a * bp

    return {
        "pkb": pkb.astype(bfloat16),
        "pkf": pkf.astype(np.float32),
    }


def kernel(x, E_id_emb, T_D_emb, T_W_emb, weights_pool, bias_pool, Wdg, a, b,
           fc1_w, fc1_b, fc2_w, fc2_b, fc3_w, fc3_b):
    global LAST_EXEC_NS
    from concourse.bass_utils import run_bass_kernel_spmd

    x = np.asarray(x, np.float32)
    E = np.asarray(E_id_emb, np.float32)
    TD = np.asarray(T_D_emb, np.float32)
    TW = np.asarray(T_W_emb, np.float32)
    Wp = np.asarray(weights_pool, np.float32)
    bp = np.asarray(bias_pool, np.float32)
    Wdg_ = np.asarray(Wdg, np.float32)
    a_ = float(np.asarray(a).reshape(-1)[0])
    b_ = float(np.asarray(b).reshape(-1)[0])
    args = (x, E, TD, TW, Wp, bp, Wdg_, a_, b_,
            np.asarray(fc1_w, np.float32), np.asarray(fc1_b, np.float32),
            np.asarray(fc2_w, np.float32), np.asarray(fc2_b, np.float32),
            np.asarray(fc3_w, np.float32), np.asarray(fc3_b, np.float32))

    key = os.environ.get("KERNEL_PHASES", "6")
    if key not in _NC_CACHE:
        _NC_CACHE[key] = _build_nc()
    nc = _NC_CACHE[key]

    in_maps = [_prep_core_inputs(c, *args) for c in range(NCORES)]
    trace = bool(int(os.environ.get("BASS_KERNEL_TRACE", "0")))
    if trace:
        try:
            import profile_hook  # noqa: F401
        except ImportError:
            pass
    res = run_bass_kernel_spmd(nc, in_maps, core_ids=list(range(NCORES)),
                               trace=trace)
    LAST_EXEC_NS = res.exec_time_ns
    out = np.concatenate([res.results[c]["out"] for c in range(NCORES)], axis=0)
    return np.ascontiguousarray(out.astype(np.float32))


# revision 20
# speedup vs baseline: 1.2594x; 1.2594x over previous
"""AGCN Trainium2 kernel — 8-core data-parallel over batch.

Math (per batch b, N=1000 nodes, Din=32, Dout=64, D=16, K=2):
  AS  = relu(tanh(3 E E^T))                       [N,N] static, symmetric
  d   = rowsum(AS)^-1/2 ; AG = d AS d
  ho  = AS @ x[b]; DF = MLP(ho); Et = E*TD*TW; DE = tanh(3 Et DF)
  DA  = relu(tanh(3 DE DE^T))                     [N,N] per batch
  out = a*(einsum over per-node weights E@Wpool applied to [x, AG@x]) + a*E@bias_pool
      + b*(DA @ x) @ Wdg

Each core handles 4 batches; AS/weights replicated per core.
v2 notes:
  - inputs packed into 2 DRAM tensors (3+1 descriptors) to kill
    DIRECT2D descriptor-issue serialization at startup.
  - E64 (E rows broadcast 64-wide for the agconv yt products) is built
    with ONE broadcast DMA from the packed DRAM input at kernel start;
    the d factor is no longer folded into E but applied on the xg
    drain via a d128 broadcast tile (tiny DRAM round-trip of d only).
  - AS / bias / S matmuls sliced to their true contraction depth K=16
    (no zero-row streaming); everything bf16 on the PE.
  - MLP sigmoids replaced by tanh identities (sigmoid(z) =
    0.5*tanh(z/2)+0.5) with affine corrections folded into the next
    layer's weights host-side -> single ACT table, no table loads.
  - output DMA: 2 descriptors per batch instead of 8, spread across
    engines.
"""

import os
import sys

for _p in ("/opt/trn_rl_repo", "/root/.axon_site/_ro/trn_rl_repo"):
    if os.path.isdir(_p) and _p not in sys.path:
        sys.path.append(_p)

from contextlib import ExitStack

import ml_dtypes
import numpy as np

import concourse.bass as bass
import concourse.tile as tile
from concourse import bacc, mybir
from concourse.masks import make_identity

BF16 = mybir.dt.bfloat16
F32 = mybir.dt.float32
I32 = mybir.dt.int32
AF = mybir.ActivationFunctionType
OP = mybir.AluOpType
bfloat16 = ml_dtypes.bfloat16

NCORES = 8
NB = 4  # batches per core
N = 1000
NP = 1024
NT = 8  # node tiles of 128
ALPHA = 3.0
RSQRT_MAGIC = 0x5F3759DF

LAST_EXEC_NS = None
_NC_CACHE = {}

# packed bf16 input layout (columns)
C_XQ = 0          # [128, 8, 128]
C_XT2 = 1024      # [128, 4, 1024]
C_ETB = 5120      # [128, 1024]
C_TDT = 6144
C_TWT = 7168
C_WPT = 8192      # [128, 8, 64]
C_FC1 = 8704      # [128, 32]
C_FC2 = 8736      # [128, 16]
C_FC3 = 8752      # [128, 16]
C_BWDG = 8768     # [128, 64]
C_AB = 8832       # [16, 64] (rows 0-15)
C16 = 8896
# packed f32 input layout
F_B1 = 0
F_B2 = 1
F_B3 = 2
F_DM = 3          # [128, 8]
C32 = 12

if os.environ.get("KERNEL_LDW_OPT", "0") == "1":
    from concourse import bass_utils as _bu

    _orig_run_command = _bu.run_command

    def _run_command_ldw(argv, **kw):
        argv = [a.replace("--enable-ldw-opt=false", "--enable-ldw-opt=true")
                if isinstance(a, str) else a for a in argv]
        return _orig_run_command(argv, **kw)

    _bu.run_command = _run_command_ldw


def _build_body(nc, tc, ctx, t):
    """t: dict of dram tensor handles."""
    P = 128
    PHASES = int(os.environ.get("KERNEL_PHASES", "6"))

    pers = ctx.enter_context(tc.tile_pool(name="pers", bufs=1))
    work = ctx.enter_context(tc.tile_pool(name="work", bufs=3))
    da_p = ctx.enter_context(tc.tile_pool(name="da", bufs=6))
    yt_p = ctx.enter_context(tc.tile_pool(name="yt", bufs=6))
    # PSUM static budget: 8 banks = 16KB/partition.
    # ps_b tag "big" bufs=2 -> 4 banks: AS, MLP, S tiles (sequential uses)
    # ps_a bufs=1, tag acc1 (ho -> dg -> tr) 2 banks, tag acc2 (xg -> ag) 2 banks
    ps_b = ctx.enter_context(tc.tile_pool(name="ps_b", bufs=2, space="PSUM"))
    ps_a = ctx.enter_context(tc.tile_pool(name="ps_a", bufs=1, space="PSUM"))
    dram = ctx.enter_context(tc.tile_pool(name="dram", bufs=1, space="DRAM"))

    # ---- persistent SBUF tiles ----
    pkb = pers.tile([P, C16], BF16)
    pkf = pers.tile([P, C32], F32)
    xq = pkb[:, C_XQ:C_XQ + 1024].rearrange("p (t c) -> p t c", t=NT)
    xT2 = pkb[:, C_XT2:C_XT2 + 4096].rearrange("p (b c) -> p b c", b=NB)
    ETb = pkb[:, C_ETB:C_ETB + 1024]
    TDT = pkb[:, C_TDT:C_TDT + 1024]
    TWT = pkb[:, C_TWT:C_TWT + 1024]
    WpT = pkb[:, C_WPT:C_WPT + 512].rearrange("p (t c) -> p t c", t=NT)
    fc1T = pkb[:, C_FC1:C_FC1 + 32]
    fc2T = pkb[:, C_FC2:C_FC2 + 16]
    fc3T = pkb[:, C_FC3:C_FC3 + 16]
    bWdg4 = pkb[:, C_BWDG:C_BWDG + 64]
    ab16 = pkb[0:16, C_AB:C_AB + 64]
    b1p = pkf[:, F_B1:F_B1 + 1]
    b2p = pkf[:, F_B2:F_B2 + 1]
    b3p = pkf[:, F_B3:F_B3 + 1]
    dmask = pkf[:, F_DM:F_DM + 8]

    E64 = pers.tile([P, NT, NP], BF16)        # chunk c: rows 0-63 E[2c], 64-127 E[2c+1]
    dxq = pers.tile([P, NT, P], BF16)
    AS = pers.tile([P, NT, NP], BF16)         # AS row-tiles
    hoT = pers.tile([P, NP], BF16)
    h1 = pers.tile([P, NP], BF16)
    h2 = pers.tile([P, NP], BF16)
    Et = pers.tile([P, NP], BF16)
    EtDF = pers.tile([P, NP], BF16)
    DET = pers.tile([P, NP], BF16)
    dgT = pers.tile([P, NP], BF16)            # x_dg2^T packed (4b*32c)
    xg_sb = pers.tile([P, NP], BF16)          # staging for d*(AS@(d*x)) drains
    d128 = pers.tile([P, NP], BF16)           # d broadcast along free dim
    ones1 = pers.tile([1, P], BF16)
    agT = pers.tile([P, 2, NP], BF16)         # final out^T per batch-pair
    I128f = pers.tile([P, P], F32)
    I128b = pers.tile([P, P], BF16)
    rs_col = pers.tile([P, NT], F32)          # rowsum(AS) per node tile
    biasT2 = pers.tile([P, NP], F32)
    d_col = pers.tile([P, NT], F32)
    magic = pers.tile([P, NT], I32)

    # ---- input DMAs: ETb gates AS, xq gates ho -- both early, balanced
    # across the two hwdge rings; xT2/weights land later.
    nc.sync.dma_start(out=pkb[:, C_ETB:C_ETB + 1024],
                      in_=t["pkb"].ap()[:, C_ETB:C_ETB + 1024])
    nc.sync.dma_start(out=pkb[:, 0:1024], in_=t["pkb"].ap()[:, 0:1024])
    nc.sync.dma_start(out=pkb[:, 1024:3072], in_=t["pkb"].ap()[:, 1024:3072])
    nc.scalar.dma_start(out=pkb[:, 3072:5120], in_=t["pkb"].ap()[:, 3072:5120])
    nc.scalar.dma_start(out=pkb[:, C_TDT:C16], in_=t["pkb"].ap()[:, C_TDT:C16])
    nc.gpsimd.dma_start(out=pkf, in_=t["pkf"].ap())

    # E64 broadcast from the packed DRAM ETb region, 8 x 256KB descriptors
    # alternating between the two hwdge rings. Ring FIFO order places these
    # after the input loads; chunk 0 lands first, matching z0's consumption
    # order. dst (p, c, col) <- pkb_dram[2c + p//64, C_ETB + col]
    pk = t["pkb"].ap()
    for cp in range(4):
        for hf in range(2):
            e_src = bass.AP(
                tensor=pk.tensor,
                offset=pk.offset + C_ETB + (4 * cp + hf) * C16,
                ap=[[0, 64], [2 * C16, 2], [1, NP]])
            eng = nc.sync if cp % 2 == 0 else nc.scalar
            eng.dma_start(out=E64[64 * hf:64 * hf + 64, 2 * cp:2 * cp + 2, :],
                          in_=e_src)

    make_identity(nc, I128f)
    make_identity(nc, I128b)
    nc.vector.memset(magic, RSQRT_MAGIC)
    nc.vector.memset(ones1, 1.0)

    out_d = t["out"]
    oa = out_d.ap()
    dma_engs = [nc.sync, nc.scalar, nc.gpsimd]

    # ================= Phase 1: AS = relu(tanh(3 E E^T)) =================
    # K=16 contraction (single E^T replica rows 0-15); rowsums fused into
    # the per-tile loop so the d-chain starts as early as possible.
    for mt in range(NT):
        ps = ps_b.tile([P, NP], F32, tag="big")
        for r in range(2):
            nc.tensor.matmul(
                ps[:, r * 512:(r + 1) * 512],
                lhsT=ETb[0:16, mt * P:(mt + 1) * P],
                rhs=ETb[0:16, r * 512:(r + 1) * 512],
                start=True, stop=True,
            )
        nc.scalar.activation(AS[:, mt, :], ps, AF.Tanh, scale=ALPHA)
        nc.vector.tensor_scalar_max(AS[:, mt, :], AS[:, mt, :], 0.0)
        nc.vector.tensor_reduce(rs_col[:, mt:mt + 1], AS[:, mt, :],
                                mybir.AxisListType.X, OP.add)

    # ho matmuls depend only on AS -- keep PE busy during the d-chain
    ho_ps = ps_a.tile([P, NP], F32, tag="acc1")
    xg_ps = ps_a.tile([P, NP], F32, tag="acc2")
    for vt in range(NT):
        for r in range(2):
            nc.tensor.matmul(
                ho_ps[:, r * 512:(r + 1) * 512],
                lhsT=xq[:, vt, :],
                rhs=AS[:, vt, r * 512:(r + 1) * 512],
                start=(vt == 0), stop=(vt == NT - 1),
            )
    # hoT drain early so m1 can run right after ho
    nc.vector.tensor_copy(hoT, ho_ps)

    # rsqrt via magic-number + 2 Newton iterations (avoids ACT table switch)
    s_sb = work.tile([P, NT], F32, tag="dtmp")
    nc.vector.tensor_scalar_max(s_sb, rs_col, 1e-6)
    sh = work.tile([P, NT], I32, tag="dtmp_i")
    nc.vector.tensor_scalar(sh, s_sb.bitcast(I32), 1, 0, OP.logical_shift_right, OP.bypass)
    y0 = work.tile([P, NT], I32, tag="dtmp_y")
    nc.vector.tensor_tensor(y0, magic, sh, OP.subtract)
    yf = y0.bitcast(F32)
    cur = yf
    for it in range(2):
        t1 = work.tile([P, NT], F32, tag=f"nt1_{it}")
        nc.vector.tensor_tensor(t1, cur, cur, OP.mult)
        nc.vector.tensor_tensor(t1, t1, s_sb, OP.mult)
        nc.vector.tensor_scalar(t1, t1, -0.5, 1.5, OP.mult, OP.add)
        t2 = work.tile([P, NT], F32, tag=f"nt2_{it}")
        nc.vector.tensor_tensor(t2, cur, t1, OP.mult)
        cur = t2
    nc.vector.tensor_tensor(d_col, cur, dmask, OP.mult)  # mask kills padded nodes

    # dxq = d * x (token-major, per-partition scalar)
    for mt in range(NT):
        nc.vector.tensor_scalar_mul(dxq[:, mt, :], xq[:, mt, :], d_col[:, mt:mt + 1])

    # ===================== MLP m1 (tanh-folded) =====================
    # sigmoid(z) = 0.5*tanh(z/2)+0.5; affine parts folded into fc2/fc3
    # host-side, so only Tanh is ever used (no ACT table switches).
    m1_ps = ps_b.tile([P, NP], F32, tag="big")
    for j in range(NB):
        for r in range(2):
            nc.tensor.matmul(
                m1_ps[32 * j:32 * j + 32, r * 512:(r + 1) * 512],
                lhsT=fc1T[32 * j:32 * j + 32, :],
                rhs=hoT[32 * j:32 * j + 32, r * 512:(r + 1) * 512],
                start=True, stop=True, tile_position=(32 * j, 32 * j),
            )
    nc.scalar.activation(h1, m1_ps, AF.Tanh, bias=b1p[:, 0:1], scale=0.5)

    # biasT2[0:64]=a*(E@bias_pool)^T, [64:128]=same (both batches of a pair)
    bias_ps = ps_b.tile([P, NP], F32, tag="big")
    for half in range(2):
        for tch in range(2):
            nc.tensor.matmul(
                bias_ps[64 * half:64 * half + 64, tch * 512:(tch + 1) * 512],
                lhsT=ab16, rhs=ETb[0:16, tch * 512:(tch + 1) * 512],
                start=True, stop=True, tile_position=(0, 64 * half),
            )

    # d128: d as a free-dim row broadcast to all 128 partitions. A tiny
    # DRAM round-trip (2KB) turns the per-partition d_col into a [1,1024]
    # row: write [128,8] bf16 row-major, read back with a gather AP
    # (n = mt*128+p -> offset p*8+mt); then a ones[1,128] x row matmul
    # broadcasts it to every partition.
    d_colb = work.tile([P, NT], BF16, tag="d8")
    nc.vector.tensor_copy(d_colb, d_col)
    dcol_d = dram.tile([P, NT], BF16)
    nc.gpsimd.dma_start(out=dcol_d, in_=d_colb)
    d8r = work.tile([1, NP], BF16, tag="d8r")
    dsrc = bass.AP(tensor=dcol_d.tensor, offset=dcol_d.offset,
                   ap=[[1, NT], [NT, P]])
    nc.gpsimd.dma_start(out=d8r.rearrange("o (t p) -> o t p", t=NT), in_=dsrc)
    d_bc = ps_b.tile([P, NP], F32, tag="big")
    for r in range(2):
        nc.tensor.matmul(d_bc[:, r * 512:(r + 1) * 512], lhsT=ones1,
                         rhs=d8r[0:1, r * 512:(r + 1) * 512],
                         start=True, stop=True)
    nc.vector.tensor_copy(d128, d_bc)

    if PHASES < 2:
        probe = work.tile([P, NT, 64], F32, tag="probe")
        for mt in range(NT):
            nc.vector.tensor_copy(probe[:, mt, :], AS[:, mt, 0:64])
        for nt_i in range(NT):
            nc.sync.dma_start(out=out_d.ap()[0, nt_i * 125:nt_i * 125 + 125, :],
                              in_=probe[0:125, nt_i, :])
        nc.sync.dma_start(out=out_d.ap()[1, 0:128, 0:8],
                          in_=d_col)
        p2 = work.tile([P, 64], F32, tag="probe2")
        nc.vector.tensor_copy(p2, E64[:, 0, 0:64])
        nc.sync.dma_start(out=out_d.ap()[1, 128:256, 0:64], in_=p2)
        p3 = work.tile([P, 64], F32, tag="probe3")
        nc.vector.tensor_copy(p3, d128[:, 0:64])
        nc.sync.dma_start(out=out_d.ap()[1, 256:384, 0:64], in_=p3)
        return

    # ============ Phase 2: xg = d * (AS @ (d*x))^T ============
    for vt in range(NT):
        for r in range(2):
            nc.tensor.matmul(
                xg_ps[:, r * 512:(r + 1) * 512],
                lhsT=dxq[:, vt, :],
                rhs=AS[:, vt, r * 512:(r + 1) * 512],
                start=(vt == 0), stop=(vt == NT - 1),
            )

    # ===================== MLP m2/m3 =====================
    m2_ps = ps_b.tile([P, NP], F32, tag="big")
    for j in range(NB):
        for r in range(2):
            nc.tensor.matmul(
                m2_ps[32 * j:32 * j + 16, r * 512:(r + 1) * 512],
                lhsT=fc2T[32 * j:32 * j + 32, :],
                rhs=h1[32 * j:32 * j + 32, r * 512:(r + 1) * 512],
                start=True, stop=True, tile_position=(32 * j, 32 * j),
            )
    nc.scalar.activation(h2, m2_ps, AF.Tanh, bias=b2p[:, 0:1], scale=0.5)
    m3_ps = ps_b.tile([P, NP], F32, tag="big")
    for j in range(NB):
        for r in range(2):
            nc.tensor.matmul(
                m3_ps[32 * j:32 * j + 16, r * 512:(r + 1) * 512],
                lhsT=fc3T[32 * j:32 * j + 16, :],
                rhs=h2[32 * j:32 * j + 16, r * 512:(r + 1) * 512],
                start=True, stop=True, tile_position=(32 * j, 32 * j),
            )

    # drains not on the DET critical path
    nc.vector.tensor_tensor(xg_sb, xg_ps, d128, OP.mult)
    for j in range(NB):
        nc.gpsimd.dma_start(out=xT2[32:64, j, :], in_=xg_sb[32 * j:32 * j + 32, :])
        nc.gpsimd.dma_start(out=xT2[96:128, j, :], in_=xg_sb[32 * j:32 * j + 32, :])
    nc.vector.tensor_copy(biasT2, bias_ps)
    nc.vector.tensor_tensor(Et, TDT, TWT, OP.mult)
    nc.vector.tensor_tensor(Et, Et, ETb, OP.mult)

    if PHASES < 4:
        nc.vector.scalar_tensor_tensor(EtDF, m3_ps, b3p[:, 0:1], Et, OP.add, OP.mult)
        nc.scalar.activation(DET, EtDF, AF.Tanh, scale=ALPHA)
        p2 = work.tile([P, 64], F32, tag="probe2")
        nc.vector.tensor_copy(p2, DET[:, 0:64])
        nc.sync.dma_start(out=out_d.ap()[0, 0:128, :], in_=p2)
        p3 = work.tile([P, 64], F32, tag="probe3")
        nc.vector.tensor_copy(p3, xT2[:, 0, 0:64])
        nc.sync.dma_start(out=out_d.ap()[1, 0:128, :], in_=p3)
        return

    # ===== Phase 4a: agconv z chunks. For pair 0 (inline, pre-S) the
    # EtDF/DET ops are slotted after chunk 1's yt builds so DET is ready
    # the moment the PE finishes the z0 block.
    def z_chunks(zp, ag_ps, cs, hook=None):
        for c in cs:
            for bb in range(2):
                j = 2 * zp + bb
                yt = yt_p.tile([P, NP], BF16, tag="yt")
                nc.vector.tensor_tensor(yt, xT2[:, j, :], E64[:, c, :], OP.mult)
                for tch in range(2):
                    nc.tensor.matmul(
                        ag_ps[64 * bb:64 * bb + 64, tch, :],
                        lhsT=WpT[:, c, :],
                        rhs=yt[:, tch * 512:(tch + 1) * 512],
                        start=(c == 0), stop=(c == NT - 1),
                        tile_position=(0, 64 * bb),
                        skip_group_check=True,
                    )
            if hook is not None and c == 1:
                hook()

    def ag_drain(zp, ag_ps):
        nc.vector.tensor_tensor(
            agT[:, zp, :].rearrange("p (a b) -> p a b", a=2),
            ag_ps, biasT2.rearrange("p (a b) -> p a b", a=2), OP.add)

    def det_hook():
        # EtDF = (DF + b3) * Et ; DE^T = tanh(3 EtDF)
        nc.vector.scalar_tensor_tensor(EtDF, m3_ps, b3p[:, 0:1], Et, OP.add, OP.mult)
        nc.scalar.activation(DET, EtDF, AF.Tanh, scale=ALPHA)

    ag0 = ps_a.tile([P, 2, 512], F32, tag="acc2")
    z_chunks(0, ag0, range(NT), hook=det_hook)
    ag_drain(0, ag0)

    # ===== Phase 4b: S / DA / x_dg2 pipeline, z1 interleaved on mt 4-7 =====
    dg_ps = ps_a.tile([P, NP], F32, tag="acc1")
    ag1 = None
    for mt in range(NT):
        s_tiles = []
        da_tiles = []
        for half in range(2):
            # two S tiles in flight (psum big pool bufs=2); K=16 slices of DET
            for bb in range(2):
                j = 2 * half + bb
                s_ps = ps_b.tile([P, NP], F32, tag="big")
                s_tiles.append(s_ps)
                for r in range(2):
                    nc.tensor.matmul(
                        s_ps[:, r * 512:(r + 1) * 512],
                        lhsT=DET[32 * j:32 * j + 16, mt * P:(mt + 1) * P],
                        rhs=DET[32 * j:32 * j + 16, r * 512:(r + 1) * 512],
                        start=True, stop=True, tile_position=(32 * j, 0),
                    )
            for bb in range(2):
                j = 2 * half + bb
                da_t = da_p.tile([P, NP], BF16, tag="da")
                da_tiles.append(da_t)
                nc.scalar.activation(da_t, s_tiles[j], AF.Tanh, scale=ALPHA)
                nc.vector.tensor_scalar_max(da_t, da_t, 0.0)
            for bb in range(2):
                j = 2 * half + bb
                for r in range(2):
                    nc.tensor.matmul(
                        dg_ps[32 * j:32 * j + 32, r * 512:(r + 1) * 512],
                        lhsT=xq[:, mt, 32 * j:32 * j + 32],
                        rhs=da_tiles[j][:, r * 512:(r + 1) * 512],
                        start=(mt == 0), stop=(mt == NT - 1),
                        tile_position=(0, 32 * j),
                    )
        if mt >= 4:
            if ag1 is None:
                ag1 = ps_a.tile([P, 2, 512], F32, tag="acc2")
            z_chunks(1, ag1, (2 * (mt - 4), 2 * (mt - 4) + 1))
    ag_drain(1, ag1)
    nc.vector.tensor_copy(dgT, dg_ps)

    # ====== Phase 6: transpose to token-major + dgconv fold + DMA out ======
    # bf16 transposes (1 cycle/row) into a bf16 psum; dg fold into a f32
    # psum; fold the two on the DVE during the drain.
    for b in range(NB):
        pair, bb = b // 2, b % 2
        trt = ps_a.tile([P, NT, 64], BF16, tag="acc1" if b % 2 == 0 else "acc2")
        trd = ps_b.tile([P, NT, 64], F32, tag="big")
        for nt_i in range(NT):
            nc.tensor.matmul(
                trt[:, nt_i, :],
                lhsT=agT[64 * bb:64 * bb + 64, pair, nt_i * P:(nt_i + 1) * P],
                rhs=I128b[64 * bb:64 * bb + 64, 64 * bb:64 * bb + 64],
                is_transpose=True, start=True, stop=True,
            )
            nc.tensor.matmul(
                trd[:, nt_i, :],
                lhsT=dgT[32 * b:32 * b + 32, nt_i * P:(nt_i + 1) * P],
                rhs=bWdg4[32 * b:32 * b + 32, :],
                start=True, stop=True,
                tile_position=(32 * b, 0),
                skip_group_check=True,
            )
        trt_sb = work.tile([P, NT, 64], BF16, tag=f"trt_sb{b % 2}")
        nc.scalar.copy(trt_sb, trt)
        tr_sb = work.tile([P, NT, 64], F32, tag=f"tr_sb{b % 2}")
        nc.vector.tensor_tensor(tr_sb, trd, trt_sb, OP.add)
        # 2 output descriptors per batch: tiles 0-6 in one 3D AP, tile 7 alone
        o_ap = bass.AP(tensor=oa.tensor, offset=oa.offset + b * N * 64,
                       ap=[[64, 128], [128 * 64, 7], [1, 64]])
        dma_engs[b % 2].dma_start(out=o_ap, in_=tr_sb[:, 0:7, :])
        o_ap7 = bass.AP(tensor=oa.tensor,
                        offset=oa.offset + b * N * 64 + 896 * 64,
                        ap=[[64, 104], [1, 64]])
        dma_engs[(b + 1) % 2].dma_start(out=o_ap7, in_=tr_sb[0:104, 7, :])


def _build_nc():
    nc = bacc.Bacc("TRN2", target_bir_lowering=False, debug=False,
                   num_devices=NCORES)
    P = 128
    t = {}
    t["pkb"] = nc.dram_tensor("pkb", [P, C16], BF16, kind="ExternalInput")
    t["pkf"] = nc.dram_tensor("pkf", [P, C32], F32, kind="ExternalInput")
    t["out"] = nc.dram_tensor("out", [NB, N, 64], F32, kind="ExternalOutput")

    with tile.TileContext(nc) as tc:
        with ExitStack() as ctx:
            _build_body(nc, tc, ctx, t)
    nc.finalize()
    return nc


def _prep_core_inputs(core, x, E, TD, TW, Wp, bp, Wdg, a, b,
                      fc1_w, fc1_b, fc2_w, fc2_b, fc3_w, fc3_b):
    P = 128
    bs = slice(NB * core, NB * (core + 1))
    xp = np.zeros((NB, NP, 32), np.float32)
    xp[:, :N] = x[bs]
    Ep = np.zeros((NP, 16), np.float32)
    Ep[:N] = E

    pkb = np.zeros((P, C16), np.float32)
    pkf = np.zeros((P, C32), np.float32)

    xq = pkb[:, C_XQ:C_XQ + 1024].reshape(P, NT, P)
    for ti in range(NT):
        blk = xp[:, ti * P:(ti + 1) * P, :]          # [4,128,32]
        xq[:, ti, :] = blk.transpose(1, 0, 2).reshape(P, P)
    xT2 = pkb[:, C_XT2:C_XT2 + 4096].reshape(P, NB, NP)
    xT = xp.transpose(2, 0, 1)                        # [32, 4, 1024]
    xT2[0:32] = xT
    xT2[64:96] = xT

    # MLP folding: sigmoid(z) = 0.5*tanh(z/2)+0.5
    #  h1 = 0.5*t1+0.5, t1 = tanh(0.5*(fc1 ho + b1))
    #  z2 = fc2 h1 + b2 = (0.5 fc2) t1 + (b2 + 0.5 rowsum(fc2))
    #  h2 = 0.5*t2+0.5, t2 = tanh(0.5*z2)
    #  DF = fc3 h2 + b3 = (0.5 fc3) t2 + (b3 + 0.5 rowsum(fc3))
    fc2h = 0.5 * fc2_w
    b2f = fc2_b + 0.5 * fc2_w.sum(axis=1)
    fc3h = 0.5 * fc3_w
    b3f = fc3_b + 0.5 * fc3_w.sum(axis=1)

    for j in range(NB):
        r0 = 32 * j
        pkb[r0:r0 + 16, C_ETB:C_ETB + 1024] = Ep.T
        pkb[r0:r0 + 16, C_TDT:C_TDT + 1024][:, :N] = TD[NB * core + j].T
        pkb[r0:r0 + 16, C_TWT:C_TWT + 1024][:, :N] = TW[NB * core + j].T
        pkb[r0:r0 + 32, C_FC1:C_FC1 + 32] = fc1_w.T
        pkb[r0:r0 + 32, C_FC2:C_FC2 + 16] = fc2h.T
        pkb[r0:r0 + 16, C_FC3:C_FC3 + 16] = fc3h.T
        pkb[r0:r0 + 32, C_BWDG:C_BWDG + 64] = b * Wdg
        pkf[r0:r0 + 32, F_B1] = 0.5 * fc1_b
        pkf[r0:r0 + 16, F_B2] = 0.5 * b2f
        pkf[r0:r0 + 16, F_B3] = b3f

    for mt in range(NT):
        for p in range(P):
            pkf[p, F_DM + mt] = 1.0 if mt * P + p < N else 0.0

    WpT = pkb[:, C_WPT:C_WPT + 512].reshape(P, NT, 64)
    for c in range(NT):
        for h in range(2):
            d = 2 * c + h
            WpT[64 * h:64 * h + 32, c, :] = a * Wp[d, 0]
            WpT[64 * h + 32:64 * h + 64, c, :] = a * Wp[d, 1]

    pkb[0:16, C_AB:C_AB + 64] = a * bp

    return {
        "pkb": pkb.astype(bfloat16),
        "pkf": pkf.astype(np.float32),
    }


def kernel(x, E_id_emb, T_D_emb, T_W_emb, weights_pool, bias_pool, Wdg, a, b,
           fc1_w, fc1_b, fc2_w, fc2_b, fc3_w, fc3_b):
    global LAST_EXEC_NS
    from concourse.bass_utils import run_bass_kernel_spmd

    x = np.asarray(x, np.float32)
    E = np.asarray(E_id_emb, np.float32)
    TD = np.asarray(T_D_emb, np.float32)
    TW = np.asarray(T_W_emb, np.float32)
    Wp = np.asarray(weights_pool, np.float32)
    bp = np.asarray(bias_pool, np.float32)
    Wdg_ = np.asarray(Wdg, np.float32)
    a_ = float(np.asarray(a).reshape(-1)[0])
    b_ = float(np.asarray(b).reshape(-1)[0])
    args = (x, E, TD, TW, Wp, bp, Wdg_, a_, b_,
            np.asarray(fc1_w, np.float32), np.asarray(fc1_b, np.float32),
            np.asarray(fc2_w, np.float32), np.asarray(fc2_b, np.float32),
            np.asarray(fc3_w, np.float32), np.asarray(fc3_b, np.float32))

    key = os.environ.get("KERNEL_PHASES", "6")
    if key not in _NC_CACHE:
        _NC_CACHE[key] = _build_nc()
    nc = _NC_CACHE[key]

    in_maps = [_prep_core_inputs(c, *args) for c in range(NCORES)]
    trace = bool(int(os.environ.get("BASS_KERNEL_TRACE", "0")))
    if trace:
        try:
            import profile_hook  # noqa: F401
        except ImportError:
            pass
    res = run_bass_kernel_spmd(nc, in_maps, core_ids=list(range(NCORES)),
                               trace=trace)
    LAST_EXEC_NS = res.exec_time_ns
    out = np.concatenate([res.results[c]["out"] for c in range(NCORES)], axis=0)
    return np.ascontiguousarray(out.astype(np.float32))


# revision 22
# speedup vs baseline: 1.3331x; 1.0586x over previous
"""AGCN Trainium2 kernel — 8-core data-parallel over batch.

Math (per batch b, N=1000 nodes, Din=32, Dout=64, D=16, K=2):
  AS  = relu(tanh(3 E E^T))                       [N,N] static, symmetric
  d   = rowsum(AS)^-1/2 ; AG = d AS d
  ho  = AS @ x[b]; DF = MLP(ho); Et = E*TD*TW; DE = tanh(3 Et DF)
  DA  = relu(tanh(3 DE DE^T))                     [N,N] per batch
  out = a*(einsum over per-node weights E@Wpool applied to [x, AG@x]) + a*E@bias_pool
      + b*(DA @ x) @ Wdg

Each core handles 4 batches; AS/weights replicated per core.
v2 notes:
  - inputs packed into 2 DRAM tensors (3+1 descriptors) to kill
    DIRECT2D descriptor-issue serialization at startup.
  - E64 (E rows broadcast 64-wide for the agconv yt products) is built
    with ONE broadcast DMA from the packed DRAM input at kernel start;
    the d factor is no longer folded into E but applied on the xg
    drain via a d128 broadcast tile (tiny DRAM round-trip of d only).
  - AS / bias / S matmuls sliced to their true contraction depth K=16
    (no zero-row streaming); everything bf16 on the PE.
  - MLP sigmoids replaced by tanh identities (sigmoid(z) =
    0.5*tanh(z/2)+0.5) with affine corrections folded into the next
    layer's weights host-side -> single ACT table, no table loads.
  - output DMA: 2 descriptors per batch instead of 8, spread across
    engines.
"""

import os
import sys

for _p in ("/opt/trn_rl_repo", "/root/.axon_site/_ro/trn_rl_repo"):
    if os.path.isdir(_p) and _p not in sys.path:
        sys.path.append(_p)

from contextlib import ExitStack

import ml_dtypes
import numpy as np

import concourse.bass as bass
import concourse.tile as tile
from concourse import bacc, mybir
from concourse.masks import make_identity

BF16 = mybir.dt.bfloat16
F32 = mybir.dt.float32
I32 = mybir.dt.int32
AF = mybir.ActivationFunctionType
OP = mybir.AluOpType
bfloat16 = ml_dtypes.bfloat16

NCORES = 8
NB = 4  # batches per core
N = 1000
NP = 1024
NT = 8  # node tiles of 128
ALPHA = 3.0
RSQRT_MAGIC = 0x5F3759DF

LAST_EXEC_NS = None
_NC_CACHE = {}

# packed bf16 input layout (columns)
C_XQ = 0          # [128, 8, 128]
C_XT2 = 1024      # [128, 4, 1024]
C_ETB = 5120      # [128, 1024]
C_TDT = 6144
C_TWT = 7168
C_WPT = 8192      # [128, 8, 64]
C_FC1 = 8704      # [128, 32]
C_FC2 = 8736      # [128, 16]
C_FC3 = 8752      # [128, 16]
C_BWDG = 8768     # [128, 64]
C_AB = 8832       # [16, 64] (rows 0-15)
C16 = 8896
# packed f32 input layout
F_B1 = 0
F_B2 = 1
F_B3 = 2
F_DM = 3          # [128, 8]
C32 = 12

if os.environ.get("KERNEL_LDW_OPT", "0") == "1":
    from concourse import bass_utils as _bu

    _orig_run_command = _bu.run_command

    def _run_command_ldw(argv, **kw):
        argv = [a.replace("--enable-ldw-opt=false", "--enable-ldw-opt=true")
                if isinstance(a, str) else a for a in argv]
        return _orig_run_command(argv, **kw)

    _bu.run_command = _run_command_ldw


def _build_body(nc, tc, ctx, t):
    """t: dict of dram tensor handles."""
    P = 128
    PHASES = int(os.environ.get("KERNEL_PHASES", "6"))

    pers = ctx.enter_context(tc.tile_pool(name="pers", bufs=1))
    work = ctx.enter_context(tc.tile_pool(name="work", bufs=3))
    da_p = ctx.enter_context(tc.tile_pool(name="da", bufs=6))
    yt_p = ctx.enter_context(tc.tile_pool(name="yt", bufs=6))
    # PSUM static budget: 8 banks = 16KB/partition.
    # ps_b tag "big" bufs=2 -> 4 banks: AS, MLP, S tiles (sequential uses)
    # ps_a bufs=1, tag acc1 (ho -> dg -> tr) 2 banks, tag acc2 (xg -> ag) 2 banks
    ps_b = ctx.enter_context(tc.tile_pool(name="ps_b", bufs=2, space="PSUM"))
    ps_a = ctx.enter_context(tc.tile_pool(name="ps_a", bufs=1, space="PSUM"))
    dram = ctx.enter_context(tc.tile_pool(name="dram", bufs=1, space="DRAM"))

    # ---- persistent SBUF tiles ----
    pkb = pers.tile([P, C16], BF16)
    pkf = pers.tile([P, C32], F32)
    xq = pkb[:, C_XQ:C_XQ + 1024].rearrange("p (t c) -> p t c", t=NT)
    xT2 = pkb[:, C_XT2:C_XT2 + 4096].rearrange("p (b c) -> p b c", b=NB)
    ETb = pkb[:, C_ETB:C_ETB + 1024]
    TDT = pkb[:, C_TDT:C_TDT + 1024]
    TWT = pkb[:, C_TWT:C_TWT + 1024]
    WpT = pkb[:, C_WPT:C_WPT + 512].rearrange("p (t c) -> p t c", t=NT)
    fc1T = pkb[:, C_FC1:C_FC1 + 32]
    fc2T = pkb[:, C_FC2:C_FC2 + 16]
    fc3T = pkb[:, C_FC3:C_FC3 + 16]
    bWdg4 = pkb[:, C_BWDG:C_BWDG + 64]
    ab16 = pkb[0:16, C_AB:C_AB + 64]
    b1p = pkf[:, F_B1:F_B1 + 1]
    b2p = pkf[:, F_B2:F_B2 + 1]
    b3p = pkf[:, F_B3:F_B3 + 1]
    dmask = pkf[:, F_DM:F_DM + 8]

    E64 = pers.tile([P, NT, NP], BF16)        # chunk c: rows 0-63 E[2c], 64-127 E[2c+1]
    dxq = pers.tile([P, NT, P], BF16)
    AS = pers.tile([P, NT, NP], BF16)         # AS row-tiles
    hoT = pers.tile([P, NP], BF16)
    h1 = pers.tile([P, NP], BF16)
    h2 = pers.tile([P, NP], BF16)
    Et = pers.tile([P, NP], BF16)
    EtDF = pers.tile([P, NP], BF16)
    DET = pers.tile([P, NP], BF16)
    dgT = pers.tile([P, NP], BF16)            # x_dg2^T packed (4b*32c)
    xg_sb = pers.tile([P, NP], BF16)          # staging for d*(AS@(d*x)) drains
    d128 = pers.tile([P, NP], BF16)           # d broadcast along free dim
    ones_b128 = pers.tile([P, P], BF16)
    agT = pers.tile([P, 2, NP], BF16)         # final out^T per batch-pair
    I128f = pers.tile([P, P], F32)
    I128b = pers.tile([P, P], BF16)
    rs_col = pers.tile([P, NT], F32)          # rowsum(AS) per node tile
    biasT2 = pers.tile([P, NP], F32)
    d_col = pers.tile([P, NT], F32)
    magic = pers.tile([P, NT], I32)

    # ---- input DMAs: ETb gates AS, xq gates ho -- both early, balanced
    # across the two hwdge rings; xT2/weights land later.
    nc.sync.dma_start(out=pkb[:, C_ETB:C_ETB + 1024],
                      in_=t["pkb"].ap()[:, C_ETB:C_ETB + 1024])
    nc.sync.dma_start(out=pkb[:, 0:1024], in_=t["pkb"].ap()[:, 0:1024])
    nc.sync.dma_start(out=pkb[:, 1024:3072], in_=t["pkb"].ap()[:, 1024:3072])
    nc.scalar.dma_start(out=pkb[:, 3072:5120], in_=t["pkb"].ap()[:, 3072:5120])
    nc.scalar.dma_start(out=pkb[:, C_TDT:C16], in_=t["pkb"].ap()[:, C_TDT:C16])
    nc.gpsimd.dma_start(out=pkf, in_=t["pkf"].ap())

    # E64 broadcast from the packed DRAM ETb region, 8 x 256KB descriptors
    # alternating between the two hwdge rings. Ring FIFO order places these
    # after the input loads; chunk 0 lands first, matching z0's consumption
    # order. dst (p, c, col) <- pkb_dram[2c + p//64, C_ETB + col]
    pk = t["pkb"].ap()
    for cp in range(4):
        for hf in range(2):
            e_src = bass.AP(
                tensor=pk.tensor,
                offset=pk.offset + C_ETB + (4 * cp + hf) * C16,
                ap=[[0, 64], [2 * C16, 2], [1, NP]])
            eng = nc.sync if cp % 2 == 0 else nc.scalar
            eng.dma_start(out=E64[64 * hf:64 * hf + 64, 2 * cp:2 * cp + 2, :],
                          in_=e_src)

    make_identity(nc, I128f)
    make_identity(nc, I128b)
    nc.vector.memset(magic, RSQRT_MAGIC)
    nc.vector.memset(ones_b128, 1.0)

    out_d = t["out"]
    oa = out_d.ap()
    dma_engs = [nc.sync, nc.scalar, nc.gpsimd]

    # ================= Phase 1: AS = relu(tanh(3 E E^T)) =================
    # K=16 contraction (single E^T replica rows 0-15); rowsums fused into
    # the per-tile loop so the d-chain starts as early as possible.
    for mt in range(NT):
        ps = ps_b.tile([P, NP], F32, tag="big")
        for r in range(2):
            nc.tensor.matmul(
                ps[:, r * 512:(r + 1) * 512],
                lhsT=ETb[0:16, mt * P:(mt + 1) * P],
                rhs=ETb[0:16, r * 512:(r + 1) * 512],
                start=True, stop=True,
            )
        # relu on the psum, then tanh with fused row-accumulation:
        # relu(tanh(3s)) == tanh(3*relu(s)), and accum_out yields the
        # rowsum(AS) for the d-chain with no separate reduce.
        nc.vector.tensor_scalar_max(ps, ps, 0.0)
        nc.scalar.activation(AS[:, mt, :], ps, AF.Tanh, scale=ALPHA,
                             accum_out=rs_col[:, mt:mt + 1])

    # ho matmuls depend only on AS -- keep PE busy during the d-chain
    ho_ps = ps_a.tile([P, NP], F32, tag="acc1")
    xg_ps = ps_a.tile([P, NP], F32, tag="acc2")
    for vt in range(NT):
        for r in range(2):
            nc.tensor.matmul(
                ho_ps[:, r * 512:(r + 1) * 512],
                lhsT=xq[:, vt, :],
                rhs=AS[:, vt, r * 512:(r + 1) * 512],
                start=(vt == 0), stop=(vt == NT - 1),
            )
    # hoT drain early so m1 can run right after ho
    nc.vector.tensor_copy(hoT, ho_ps)

    # rsqrt via magic-number + 2 Newton iterations (avoids ACT table switch)
    s_sb = work.tile([P, NT], F32, tag="dtmp")
    nc.vector.tensor_scalar_max(s_sb, rs_col, 1e-6)
    sh = work.tile([P, NT], I32, tag="dtmp_i")
    nc.vector.tensor_scalar(sh, s_sb.bitcast(I32), 1, 0, OP.logical_shift_right, OP.bypass)
    y0 = work.tile([P, NT], I32, tag="dtmp_y")
    nc.vector.tensor_tensor(y0, magic, sh, OP.subtract)
    yf = y0.bitcast(F32)
    cur = yf
    for it in range(2):
        t1 = work.tile([P, NT], F32, tag=f"nt1_{it}")
        nc.vector.tensor_tensor(t1, cur, cur, OP.mult)
        nc.vector.tensor_tensor(t1, t1, s_sb, OP.mult)
        nc.vector.tensor_scalar(t1, t1, -0.5, 1.5, OP.mult, OP.add)
        t2 = work.tile([P, NT], F32, tag=f"nt2_{it}")
        nc.vector.tensor_tensor(t2, cur, t1, OP.mult)
        cur = t2
    nc.vector.tensor_tensor(d_col, cur, dmask, OP.mult)  # mask kills padded nodes

    # dxq = d * x (token-major, per-partition scalar)
    for mt in range(NT):
        nc.vector.tensor_scalar_mul(dxq[:, mt, :], xq[:, mt, :], d_col[:, mt:mt + 1])

    # ===================== MLP m1 (tanh-folded) =====================
    # sigmoid(z) = 0.5*tanh(z/2)+0.5; affine parts folded into fc2/fc3
    # host-side, so only Tanh is ever used (no ACT table switches).
    m1_ps = ps_b.tile([P, NP], F32, tag="big")
    for j in range(NB):
        for r in range(2):
            nc.tensor.matmul(
                m1_ps[32 * j:32 * j + 32, r * 512:(r + 1) * 512],
                lhsT=fc1T[32 * j:32 * j + 32, :],
                rhs=hoT[32 * j:32 * j + 32, r * 512:(r + 1) * 512],
                start=True, stop=True, tile_position=(32 * j, 32 * j),
            )
    nc.scalar.activation(h1, m1_ps, AF.Tanh, bias=b1p[:, 0:1], scale=0.5)

    # biasT2[0:64]=a*(E@bias_pool)^T, [64:128]=same (both batches of a pair)
    bias_ps = ps_b.tile([P, NP], F32, tag="big")
    for half in range(2):
        for tch in range(2):
            nc.tensor.matmul(
                bias_ps[64 * half:64 * half + 64, tch * 512:(tch + 1) * 512],
                lhsT=ab16, rhs=ETb[0:16, tch * 512:(tch + 1) * 512],
                start=True, stop=True, tile_position=(0, 64 * half),
            )

    # d128: d broadcast along the free dim to all 128 partitions, all
    # on-chip: per node tile build diag(d) on the DVE (identity times
    # per-partition scalar), then ones[128,:]^T @ diag(d) puts d[n] on
    # every partition.
    d_bc = ps_b.tile([P, NP], F32, tag="big")
    for mt in range(NT):
        dgt = work.tile([P, P], BF16, tag="diag")
        nc.vector.tensor_scalar_mul(dgt, I128b, d_col[:, mt:mt + 1])
        nc.tensor.matmul(d_bc[:, mt * P:(mt + 1) * P], lhsT=ones_b128,
                         rhs=dgt, start=True, stop=True)
    nc.vector.tensor_copy(d128, d_bc)

    if PHASES < 2:
        probe = work.tile([P, NT, 64], F32, tag="probe")
        for mt in range(NT):
            nc.vector.tensor_copy(probe[:, mt, :], AS[:, mt, 0:64])
        for nt_i in range(NT):
            nc.sync.dma_start(out=out_d.ap()[0, nt_i * 125:nt_i * 125 + 125, :],
                              in_=probe[0:125, nt_i, :])
        nc.sync.dma_start(out=out_d.ap()[1, 0:128, 0:8],
                          in_=d_col)
        p2 = work.tile([P, 64], F32, tag="probe2")
        nc.vector.tensor_copy(p2, E64[:, 0, 0:64])
        nc.sync.dma_start(out=out_d.ap()[1, 128:256, 0:64], in_=p2)
        p3 = work.tile([P, 64], F32, tag="probe3")
        nc.vector.tensor_copy(p3, d128[:, 0:64])
        nc.sync.dma_start(out=out_d.ap()[1, 256:384, 0:64], in_=p3)
        return

    # ============ Phase 2: xg = d * (AS @ (d*x))^T ============
    for vt in range(NT):
        for r in range(2):
            nc.tensor.matmul(
                xg_ps[:, r * 512:(r + 1) * 512],
                lhsT=dxq[:, vt, :],
                rhs=AS[:, vt, r * 512:(r + 1) * 512],
                start=(vt == 0), stop=(vt == NT - 1),
            )

    # ===================== MLP m2/m3 =====================
    m2_ps = ps_b.tile([P, NP], F32, tag="big")
    for j in range(NB):
        for r in range(2):
            nc.tensor.matmul(
                m2_ps[32 * j:32 * j + 16, r * 512:(r + 1) * 512],
                lhsT=fc2T[32 * j:32 * j + 32, :],
                rhs=h1[32 * j:32 * j + 32, r * 512:(r + 1) * 512],
                start=True, stop=True, tile_position=(32 * j, 32 * j),
            )
    nc.scalar.activation(h2, m2_ps, AF.Tanh, bias=b2p[:, 0:1], scale=0.5)
    m3_ps = ps_b.tile([P, NP], F32, tag="big")
    for j in range(NB):
        for r in range(2):
            nc.tensor.matmul(
                m3_ps[32 * j:32 * j + 16, r * 512:(r + 1) * 512],
                lhsT=fc3T[32 * j:32 * j + 16, :],
                rhs=h2[32 * j:32 * j + 16, r * 512:(r + 1) * 512],
                start=True, stop=True, tile_position=(32 * j, 32 * j),
            )

    # drains not on the DET critical path
    nc.vector.tensor_tensor(xg_sb, xg_ps, d128, OP.mult)
    for j in range(NB):
        nc.gpsimd.dma_start(out=xT2[32:64, j, :], in_=xg_sb[32 * j:32 * j + 32, :])
        nc.gpsimd.dma_start(out=xT2[96:128, j, :], in_=xg_sb[32 * j:32 * j + 32, :])
    nc.vector.tensor_copy(biasT2, bias_ps)
    nc.vector.tensor_tensor(Et, TDT, TWT, OP.mult)
    nc.vector.tensor_tensor(Et, Et, ETb, OP.mult)

    if PHASES < 4:
        nc.vector.scalar_tensor_tensor(EtDF, m3_ps, b3p[:, 0:1], Et, OP.add, OP.mult)
        nc.scalar.activation(DET, EtDF, AF.Tanh, scale=ALPHA)
        p2 = work.tile([P, 64], F32, tag="probe2")
        nc.vector.tensor_copy(p2, DET[:, 0:64])
        nc.sync.dma_start(out=out_d.ap()[0, 0:128, :], in_=p2)
        p3 = work.tile([P, 64], F32, tag="probe3")
        nc.vector.tensor_copy(p3, xT2[:, 0, 0:64])
        nc.sync.dma_start(out=out_d.ap()[1, 0:128, :], in_=p3)
        return

    # ===== Phase 4a: agconv z chunks. For pair 0 (inline, pre-S) the
    # EtDF/DET ops are slotted after chunk 1's yt builds so DET is ready
    # the moment the PE finishes the z0 block.
    def z_chunks(zp, ag_ps, cs, hook=None):
        for c in cs:
            for bb in range(2):
                j = 2 * zp + bb
                yt = yt_p.tile([P, NP], BF16, tag="yt")
                nc.vector.tensor_tensor(yt, xT2[:, j, :], E64[:, c, :], OP.mult)
                for tch in range(2):
                    nc.tensor.matmul(
                        ag_ps[64 * bb:64 * bb + 64, tch, :],
                        lhsT=WpT[:, c, :],
                        rhs=yt[:, tch * 512:(tch + 1) * 512],
                        start=(c == 0), stop=(c == NT - 1),
                        tile_position=(0, 64 * bb),
                        skip_group_check=True,
                    )
            if hook is not None and c == 1:
                hook()

    def ag_drain(zp, ag_ps):
        nc.vector.tensor_tensor(
            agT[:, zp, :].rearrange("p (a b) -> p a b", a=2),
            ag_ps, biasT2.rearrange("p (a b) -> p a b", a=2), OP.add)

    def det_hook():
        # EtDF = (DF + b3) * Et ; DE^T = tanh(3 EtDF)
        nc.vector.scalar_tensor_tensor(EtDF, m3_ps, b3p[:, 0:1], Et, OP.add, OP.mult)
        nc.scalar.activation(DET, EtDF, AF.Tanh, scale=ALPHA)

    ag0 = ps_a.tile([P, 2, 512], F32, tag="acc2")
    z_chunks(0, ag0, range(NT), hook=det_hook)
    ag_drain(0, ag0)

    # ===== Phase 4b: S / DA / x_dg2 pipeline, z1 interleaved on mt 4-7 =====
    dg_ps = ps_a.tile([P, NP], F32, tag="acc1")
    ag1 = None
    for mt in range(NT):
        s_tiles = []
        da_tiles = []
        for half in range(2):
            # two S tiles in flight (psum big pool bufs=2); K=16 slices of DET
            for bb in range(2):
                j = 2 * half + bb
                s_ps = ps_b.tile([P, NP], F32, tag="big")
                s_tiles.append(s_ps)
                for r in range(2):
                    nc.tensor.matmul(
                        s_ps[:, r * 512:(r + 1) * 512],
                        lhsT=DET[32 * j:32 * j + 16, mt * P:(mt + 1) * P],
                        rhs=DET[32 * j:32 * j + 16, r * 512:(r + 1) * 512],
                        start=True, stop=True, tile_position=(32 * j, 0),
                    )
            for bb in range(2):
                j = 2 * half + bb
                da_t = da_p.tile([P, NP], BF16, tag="da")
                da_tiles.append(da_t)
                nc.scalar.activation(da_t, s_tiles[j], AF.Tanh, scale=ALPHA)
                nc.vector.tensor_scalar_max(da_t, da_t, 0.0)
            for bb in range(2):
                j = 2 * half + bb
                for r in range(2):
                    nc.tensor.matmul(
                        dg_ps[32 * j:32 * j + 32, r * 512:(r + 1) * 512],
                        lhsT=xq[:, mt, 32 * j:32 * j + 32],
                        rhs=da_tiles[j][:, r * 512:(r + 1) * 512],
                        start=(mt == 0), stop=(mt == NT - 1),
                        tile_position=(0, 32 * j),
                    )
        if mt >= 4:
            if ag1 is None:
                ag1 = ps_a.tile([P, 2, 512], F32, tag="acc2")
            z_chunks(1, ag1, (2 * (mt - 4), 2 * (mt - 4) + 1))
    ag_drain(1, ag1)
    nc.vector.tensor_copy(dgT, dg_ps)

    # ====== Phase 6: transpose to token-major + dgconv fold + DMA out ======
    # bf16 transposes (1 cycle/row) into a bf16 psum; dg fold into a f32
    # psum; fold the two on the DVE during the drain.
    for b in range(NB):
        pair, bb = b // 2, b % 2
        trt = ps_a.tile([P, NT, 64], BF16, tag="acc1" if b % 2 == 0 else "acc2")
        trd = ps_b.tile([P, NT, 64], F32, tag="big")
        for nt_i in range(NT):
            nc.tensor.matmul(
                trt[:, nt_i, :],
                lhsT=agT[64 * bb:64 * bb + 64, pair, nt_i * P:(nt_i + 1) * P],
                rhs=I128b[64 * bb:64 * bb + 64, 64 * bb:64 * bb + 64],
                is_transpose=True, start=True, stop=True,
            )
            nc.tensor.matmul(
                trd[:, nt_i, :],
                lhsT=dgT[32 * b:32 * b + 32, nt_i * P:(nt_i + 1) * P],
                rhs=bWdg4[32 * b:32 * b + 32, :],
                start=True, stop=True,
                tile_position=(32 * b, 0),
                skip_group_check=True,
            )
        trt_sb = work.tile([P, NT, 64], BF16, tag=f"trt_sb{b % 2}")
        nc.scalar.copy(trt_sb, trt)
        tr_sb = work.tile([P, NT, 64], F32, tag=f"tr_sb{b % 2}")
        nc.vector.tensor_tensor(tr_sb, trd, trt_sb, OP.add)
        # 2 output descriptors per batch: tiles 0-6 in one 3D AP, tile 7 alone
        o_ap = bass.AP(tensor=oa.tensor, offset=oa.offset + b * N * 64,
                       ap=[[64, 128], [128 * 64, 7], [1, 64]])
        dma_engs[b % 2].dma_start(out=o_ap, in_=tr_sb[:, 0:7, :])
        o_ap7 = bass.AP(tensor=oa.tensor,
                        offset=oa.offset + b * N * 64 + 896 * 64,
                        ap=[[64, 104], [1, 64]])
        dma_engs[(b + 1) % 2].dma_start(out=o_ap7, in_=tr_sb[0:104, 7, :])


def _build_nc():
    nc = bacc.Bacc("TRN2", target_bir_lowering=False, debug=False,
                   num_devices=NCORES)
    P = 128
    t = {}
    t["pkb"] = nc.dram_tensor("pkb", [P, C16], BF16, kind="ExternalInput")
    t["pkf"] = nc.dram_tensor("pkf", [P, C32], F32, kind="ExternalInput")
    t["out"] = nc.dram_tensor("out", [NB, N, 64], F32, kind="ExternalOutput")

    with tile.TileContext(nc) as tc:
        with ExitStack() as ctx:
            _build_body(nc, tc, ctx, t)
    nc.finalize()
    return nc


def _prep_core_inputs(core, x, E, TD, TW, Wp, bp, Wdg, a, b,
                      fc1_w, fc1_b, fc2_w, fc2_b, fc3_w, fc3_b):
    P = 128
    bs = slice(NB * core, NB * (core + 1))
    xp = np.zeros((NB, NP, 32), np.float32)
    xp[:, :N] = x[bs]
    Ep = np.zeros((NP, 16), np.float32)
    Ep[:N] = E

    pkb = np.zeros((P, C16), np.float32)
    pkf = np.zeros((P, C32), np.float32)

    xq = pkb[:, C_XQ:C_XQ + 1024].reshape(P, NT, P)
    for ti in range(NT):
        blk = xp[:, ti * P:(ti + 1) * P, :]          # [4,128,32]
        xq[:, ti, :] = blk.transpose(1, 0, 2).reshape(P, P)
    xT2 = pkb[:, C_XT2:C_XT2 + 4096].reshape(P, NB, NP)
    xT = xp.transpose(2, 0, 1)                        # [32, 4, 1024]
    xT2[0:32] = xT
    xT2[64:96] = xT

    # MLP folding: sigmoid(z) = 0.5*tanh(z/2)+0.5
    #  h1 = 0.5*t1+0.5, t1 = tanh(0.5*(fc1 ho + b1))
    #  z2 = fc2 h1 + b2 = (0.5 fc2) t1 + (b2 + 0.5 rowsum(fc2))
    #  h2 = 0.5*t2+0.5, t2 = tanh(0.5*z2)
    #  DF = fc3 h2 + b3 = (0.5 fc3) t2 + (b3 + 0.5 rowsum(fc3))
    fc2h = 0.5 * fc2_w
    b2f = fc2_b + 0.5 * fc2_w.sum(axis=1)
    fc3h = 0.5 * fc3_w
    b3f = fc3_b + 0.5 * fc3_w.sum(axis=1)

    for j in range(NB):
        r0 = 32 * j
        pkb[r0:r0 + 16, C_ETB:C_ETB + 1024] = Ep.T
        pkb[r0:r0 + 16, C_TDT:C_TDT + 1024][:, :N] = TD[NB * core + j].T
        pkb[r0:r0 + 16, C_TWT:C_TWT + 1024][:, :N] = TW[NB * core + j].T
        pkb[r0:r0 + 32, C_FC1:C_FC1 + 32] = fc1_w.T
        pkb[r0:r0 + 32, C_FC2:C_FC2 + 16] = fc2h.T
        pkb[r0:r0 + 16, C_FC3:C_FC3 + 16] = fc3h.T
        pkb[r0:r0 + 32, C_BWDG:C_BWDG + 64] = b * Wdg
        pkf[r0:r0 + 32, F_B1] = 0.5 * fc1_b
        pkf[r0:r0 + 16, F_B2] = 0.5 * b2f
        pkf[r0:r0 + 16, F_B3] = b3f

    for mt in range(NT):
        for p in range(P):
            pkf[p, F_DM + mt] = 1.0 if mt * P + p < N else 0.0

    WpT = pkb[:, C_WPT:C_WPT + 512].reshape(P, NT, 64)
    for c in range(NT):
        for h in range(2):
            d = 2 * c + h
            WpT[64 * h:64 * h + 32, c, :] = a * Wp[d, 0]
            WpT[64 * h + 32:64 * h + 64, c, :] = a * Wp[d, 1]

    pkb[0:16, C_AB:C_AB + 64] = a * bp

    return {
        "pkb": pkb.astype(bfloat16),
        "pkf": pkf.astype(np.float32),
    }


def kernel(x, E_id_emb, T_D_emb, T_W_emb, weights_pool, bias_pool, Wdg, a, b,
           fc1_w, fc1_b, fc2_w, fc2_b, fc3_w, fc3_b):
    global LAST_EXEC_NS
    from concourse.bass_utils import run_bass_kernel_spmd

    x = np.asarray(x, np.float32)
    E = np.asarray(E_id_emb, np.float32)
    TD = np.asarray(T_D_emb, np.float32)
    TW = np.asarray(T_W_emb, np.float32)
    Wp = np.asarray(weights_pool, np.float32)
    bp = np.asarray(bias_pool, np.float32)
    Wdg_ = np.asarray(Wdg, np.float32)
    a_ = float(np.asarray(a).reshape(-1)[0])
    b_ = float(np.asarray(b).reshape(-1)[0])
    args = (x, E, TD, TW, Wp, bp, Wdg_, a_, b_,
            np.asarray(fc1_w, np.float32), np.asarray(fc1_b, np.float32),
            np.asarray(fc2_w, np.float32), np.asarray(fc2_b, np.float32),
            np.asarray(fc3_w, np.float32), np.asarray(fc3_b, np.float32))

    key = os.environ.get("KERNEL_PHASES", "6")
    if key not in _NC_CACHE:
        _NC_CACHE[key] = _build_nc()
    nc = _NC_CACHE[key]

    in_maps = [_prep_core_inputs(c, *args) for c in range(NCORES)]
    trace = bool(int(os.environ.get("BASS_KERNEL_TRACE", "0")))
    if trace:
        try:
            import profile_hook  # noqa: F401
        except ImportError:
            pass
    res = run_bass_kernel_spmd(nc, in_maps, core_ids=list(range(NCORES)),
                               trace=trace)
    LAST_EXEC_NS = res.exec_time_ns
    out = np.concatenate([res.results[c]["out"] for c in range(NCORES)], axis=0)
    return np.ascontiguousarray(out.astype(np.float32))
